# revision 1
# baseline (speedup 1.0000x reference)
"""Ball attention (block-local attention, ball size 128) on 8 Trainium2 cores.

Reference computation (per (b,h) head, per ball of 128 consecutive tokens):
    S = Q K^T / sqrt(64);  P = softmax(S, axis=-1);  O = P V

Sharding: the 64 (b,h) heads are split 8-per-core (pure data parallel).

Per-core design (all sizes measured on HW):
  * Loads/stores use the natural ball layout (seq position on partitions).
  * Q^T/K^T via packed 2-ball PE transposes: input [128 seq, 2ball x 64d]
    -> output [2ball x 64d partitions, 128 seq]; ball b of a pair lands on
    partition half 64b. ~173ns per transpose op (PE SBUF latency bound).
  * PSUM->SBUF copies round to float32r (DVE for Q^T, ACT for K^T).
  * S^T = K Q^T as float32r "junk-pair" matmuls: rhs = [qt(ball) | qt(ball+2)]
    gives N=256 which runs at 1 cyc/row (vs 4 for fp32); half the output is
    discarded. Measured 274ns/ball, rel err ~1.7e-4 on S (float32r rounds
    mantissas; final output error ~3e-5).
  * E = exp(S^T/8) on ACT directly into SBUF; the ones-column appended to V
    provides softmax denominators through the O matmul; normalize on DVE via
    a per-partition reciprocal broadcast.
  * O matmul dtype selectable (BALL_O_DTYPE): bf16 (fast, adds ~1e-3 error),
    float32r, or float32.
"""

import os
import sys

for _p in ("/opt/trn_rl_repo",):
    if _p not in sys.path and os.path.isdir(_p):
        sys.path.insert(0, _p)

from contextlib import ExitStack

import numpy as np

import concourse.bass as bass
import concourse.mybir as mybir
import concourse.tile as tile
from concourse import bacc
from concourse._compat import with_exitstack
from concourse.masks import make_identity

B, H, N, DH = 4, 16, 8192, 64
BS = 128                 # ball size == SBUF partition count
NCORES = 8
HEADS = B * H // NCORES  # heads per core (8)
M = N // BS              # balls per head (64)

FP32 = mybir.dt.float32
FP32R = mybir.dt.float32r
BF16 = mybir.dt.bfloat16

S_MODE = os.environ.get("BALL_S_MODE", "fp32r")   # fp32r (fast, rel err 2e-4) | fp32 (rel err 1e-5)
TQK = "pe"  # DVE StreamTranspose path abandoned (AP adjacency limits)
O_MODE = os.environ.get("BALL_O_MODE", "fp32")    # bf16 | fp32r | fp32
GRP = 4
# debug bisect: 1=transposes+copies, 2=+S+exp, 3=+O, 4=full (default)
STAGE = int(os.environ.get("BALL_STAGE", "4"))


@with_exitstack
def ball_attention_kernel(
    ctx: ExitStack,
    tc: tile.TileContext,
    out_ap: bass.AP,
    q_ap: bass.AP,
    k_ap: bass.AP,
    v_ap: bass.AP,
    heads: int = HEADS,
    m: int = M,
):
    nc = tc.nc
    assert m % GRP == 0
    ngrp = m // GRP
    scale = 1.0 / float(np.sqrt(DH))
    t_dt = FP32R if S_MODE == "fp32r" else FP32
    if O_MODE == "bf16":
        e_dt = v_dt = BF16
    elif O_MODE == "fp32r":
        e_dt = v_dt = FP32R
    else:
        e_dt = v_dt = FP32

    const_pool = ctx.enter_context(tc.tile_pool(name="const", bufs=1))
    io_pool = ctx.enter_context(tc.tile_pool(name="io", bufs=2))
    t_sb_pool = ctx.enter_context(tc.tile_pool(name="t_sb", bufs=3))
    e_pool = ctx.enter_context(tc.tile_pool(name="e", bufs=2))
    r_pool = ctx.enter_context(tc.tile_pool(name="r", bufs=2))
    t_ps_pool = ctx.enter_context(tc.tile_pool(name="t_ps", bufs=2, space="PSUM"))
    s_ps_pool = ctx.enter_context(tc.tile_pool(name="s_ps", bufs=2, space="PSUM"))
    o_ps_pool = ctx.enter_context(tc.tile_pool(name="o_ps", bufs=2, space="PSUM"))

    ident = const_pool.tile([BS, BS], FP32)
    make_identity(nc, ident)

    nchunk = int(os.environ.get("BALL_NCHUNK", "2"))  # head-load split
    mc = m // nchunk
    for h in range(heads):
        # ---- loads (natural ball layout: partition = seq within ball) -----
        # spread across the three DMA paths: Q on the SP HWDGE ring, K on the
        # ACT HWDGE ring, V (+ the output store) on SWDGE.
        if TQK == "dve":
            q_sb = io_pool.tile([BS, m // 2, 4, 32], FP32, tag="q")
            k_sb = io_pool.tile([BS, m // 2, 4, 32], FP32, tag="k")
        else:
            q_sb = io_pool.tile([BS, m, DH], FP32, tag="q")
            k_sb = io_pool.tile([BS, m, DH], FP32, tag="k")
        vt = io_pool.tile([BS, m, DH + 1], v_dt, tag="vt")
        if TQK == "dve":
            # staged layout for DVE 32x32 block transposes: partition
            # (64*par + 32*db + al) <- row 32*sb+al, d 32*db+be of ball 2a+par
            qv_ = q_ap[h].rearrange(
                "(a par sb al) (db be) -> (par db al) a sb be",
                par=2, sb=4, al=32, db=2,
            )
            kv_ = k_ap[h].rearrange(
                "(a par sb al) (db be) -> (par db al) a sb be",
                par=2, sb=4, al=32, db=2,
            )
        else:
            qv_ = q_ap[h].rearrange("(mm p) d -> p mm d", p=BS)
            kv_ = k_ap[h].rearrange("(mm p) d -> p mm d", p=BS)
        vv_ = v_ap[h].rearrange("(mm p) d -> p mm d", p=BS)
        for c in range(nchunk):
            cs = slice(c * mc, (c + 1) * mc)
            if TQK == "dve":
                cs2 = slice(c * mc // 2, (c + 1) * mc // 2)
                nc.sync.dma_start(q_sb[:, cs2, :, :], qv_[:, cs2, :, :])
                nc.sync.dma_start(k_sb[:, cs2, :, :], kv_[:, cs2, :, :])
            else:
                nc.sync.dma_start(q_sb[:, cs, :], qv_[:, cs, :])
                nc.sync.dma_start(k_sb[:, cs, :], kv_[:, cs, :])
            nc.sync.dma_start(vt[:, cs, 0:DH], vv_[:, cs, :])
        nc.vector.memset(vt[:, :, DH], 1.0)
        ob = io_pool.tile([BS, m, DH], FP32, tag="ob")

        for g in range(ngrp):
            # 4 balls: m0..m0+3; junk-pairs (m0, m0+2) and (m0+1, m0+3)
            m0 = g * GRP
            qt = t_sb_pool.tile([BS, 2, BS], t_dt, tag="qt")        # [pair, seq]
            kt = t_sb_pool.tile([BS, 2, BS], t_dt, tag="kt")
            if TQK == "dve":
                # DVE 32x32 block transposes of the staged tiles give the
                # packed [64*par + d, seq] layout directly, SBUF->SBUF.
                a0 = m0 // 2
                nc.vector.transpose(qt[:, 0, :], q_sb[:, a0, :, :])
                nc.vector.transpose(kt[:, 0, :], k_sb[:, a0, :, :])
                nc.vector.transpose(qt[:, 1, :], q_sb[:, a0 + 1, :, :])
                nc.vector.transpose(kt[:, 1, :], k_sb[:, a0 + 1, :, :])
            else:
                # one bank: [qt(pair0) | kt(pair0) | qt(pair1) | kt(pair1)]
                t_ps = t_ps_pool.tile([BS, 4, BS], FP32, tag="t")
                # packed transposes: 2 balls/op; ball parity b -> partitions 64b
                nc.tensor.transpose(t_ps[:, 0, :], q_sb[:, m0 : m0 + 2, :], ident)
                nc.tensor.transpose(t_ps[:, 1, :], k_sb[:, m0 : m0 + 2, :], ident)
                nc.tensor.transpose(t_ps[:, 2, :], q_sb[:, m0 + 2 : m0 + 4, :], ident)
                nc.tensor.transpose(t_ps[:, 3, :], k_sb[:, m0 + 2 : m0 + 4, :], ident)
                # PSUM -> SBUF (+ round to fp32r): DVE takes Q^T, ACT takes K^T
                nc.vector.tensor_copy(qt, t_ps[:, 0:4:2, :])
                nc.scalar.copy(kt, t_ps[:, 1:4:2, :])
            if STAGE == 1:
                nc.vector.tensor_copy(
                    ob[:, m0 : m0 + 2, :], qt[:, :, 0:DH].bitcast(FP32)
                )
                nc.vector.tensor_copy(
                    ob[:, m0 + 2 : m0 + 4, :], kt[:, :, 0:DH].bitcast(FP32)
                )
                continue

            # S^T matmuls. qt slot layout: [pair a' = 0|1][seq], ball (2j+b)
            # at partitions 64b. junk-pair rhs = qt[64b:64b+64, :, :] (N=256).
            # Consecutive matmuls must hit different PSUM banks (same-bank
            # back-to-back PE matmul writes fault): ball j -> bank j%2,
            # slot j//2 of a 2-bank tile.
            s_ps = s_ps_pool.tile([BS, 2, 2, 2 * BS], FP32, tag="s", bufs=int(os.environ.get("BALL_SBUFS", "2")))
            e_sb = e_pool.tile([BS, GRP, BS], e_dt, tag="e")
            for j in range(GRP):
                a2, b = j >> 1, j & 1          # ball m0+j = pair a2, parity b
                lo = 64 * b
                if S_MODE == "fp32r":
                    nc.tensor.matmul(
                        s_ps[:, j % 2, j // 2, :],
                        kt[lo : lo + 64, a2, :],
                        qt[lo : lo + 64, :, :],
                        start=True,
                        stop=True,
                    )
                else:
                    nc.tensor.matmul(
                        s_ps[:, j % 2, j // 2, a2 * BS : a2 * BS + BS],
                        kt[lo : lo + 64, a2, :],
                        qt[lo : lo + 64, a2, :],
                        start=True,
                        stop=True,
                    )
            if STAGE == 15:
                for a2 in range(2):
                    nc.vector.tensor_copy(
                        ob[:, m0 + a2 * 2 : m0 + a2 * 2 + 2, :],
                        s_ps[:, :, a2, a2 * BS : a2 * BS + DH],
                    )
                continue
            # E = exp(S^T/8); good half of ball j's junk-pair output is the
            # column block of its own pair slot (a2 = j>>1 = tile slot dim).
            for a2 in range(2):
                nc.scalar.activation(
                    e_sb[:, a2 * 2 : a2 * 2 + 2, :],
                    s_ps[:, :, a2, a2 * BS : a2 * BS + BS],
                    mybir.ActivationFunctionType.Exp,
                    scale=scale,
                )

            if STAGE == 2:
                if O_MODE == "bf16":
                    for j in range(GRP):
                        nc.vector.tensor_copy(ob[:, m0 + j, :], e_sb[:, j, 0:DH])
                else:
                    nc.vector.tensor_copy(ob[:, m0 : m0 + GRP, :], e_sb[:, :, 0:DH].bitcast(FP32))
                continue

            # O_unnorm = E^T @ [V | 1]
            o_ps = o_ps_pool.tile([BS, GRP, DH + 1], FP32, tag="o")
            for j in range(GRP):
                nc.tensor.matmul(
                    o_ps[:, j, :],
                    e_sb[:, j, :],
                    vt[:, m0 + j, :],
                    start=True,
                    stop=True,
                )
            if STAGE == 3:
                nc.vector.tensor_copy(ob[:, m0 : m0 + GRP, :], o_ps[:, :, 0:DH])
                continue
            # normalize by the ones-column sums
            r_sb = r_pool.tile([BS, GRP], FP32, tag="r")
            nc.vector.reciprocal(r_sb, o_ps[:, :, DH])
            nc.vector.tensor_mul(
                ob[:, m0 : m0 + GRP, :],
                o_ps[:, :, 0:DH],
                r_sb.unsqueeze(2).broadcast_to([BS, GRP, DH]),
            )

        # ---- store -------------------------------------------------------
        nc.gpsimd.dma_start(out_ap[h].rearrange("(mm p) d -> p mm d", p=BS), ob)


def build_nc(heads: int = HEADS, m: int = M):
    nc = bacc.Bacc("TRN2", target_bir_lowering=False, debug=False, num_devices=NCORES)
    q = nc.dram_tensor("q", [heads, m * BS, DH], FP32, kind="ExternalInput").ap()
    k = nc.dram_tensor("k", [heads, m * BS, DH], FP32, kind="ExternalInput").ap()
    v = nc.dram_tensor("v", [heads, m * BS, DH], FP32, kind="ExternalInput").ap()
    o = nc.dram_tensor("out", [heads, m * BS, DH], FP32, kind="ExternalOutput").ap()
    with tile.TileContext(nc) as tc:
        ball_attention_kernel(tc, o, q, k, v, heads=heads, m=m)
    nc.compile()
    return nc


_NC_CACHE = {}


def kernel(q: np.ndarray, k: np.ndarray, v: np.ndarray) -> np.ndarray:
    from concourse.bass_utils import run_bass_kernel_spmd

    assert q.shape == (B, H, N, DH)
    if "nc" not in _NC_CACHE:
        _NC_CACHE["nc"] = build_nc()
    nc = _NC_CACHE["nc"]

    hpc = HEADS
    qf = np.ascontiguousarray(np.asarray(q, dtype=np.float32).reshape(B * H, N, DH))
    kf = np.ascontiguousarray(np.asarray(k, dtype=np.float32).reshape(B * H, N, DH))
    vf = np.ascontiguousarray(np.asarray(v, dtype=np.float32).reshape(B * H, N, DH))
    in_maps = [
        {
            "q": np.ascontiguousarray(qf[c * hpc : (c + 1) * hpc]),
            "k": np.ascontiguousarray(kf[c * hpc : (c + 1) * hpc]),
            "v": np.ascontiguousarray(vf[c * hpc : (c + 1) * hpc]),
        }
        for c in range(NCORES)
    ]
    res = run_bass_kernel_spmd(nc, in_maps, core_ids=list(range(NCORES)))
    out = np.concatenate([res.results[c]["out"] for c in range(NCORES)], axis=0)
    return out.reshape(B, H, N, DH)



# revision 4
# speedup vs baseline: 1.4626x; 1.4626x over previous
"""Ball attention (block-local attention, ball size 128) on 8 Trainium2 cores.

Reference computation (per (b,h) head, per ball of 128 consecutive tokens):
    S = Q K^T / sqrt(64);  P = softmax(S, axis=-1);  O = P V

Sharding: the 64 (b,h) heads are split 8-per-core (pure data parallel).

bf16 pipeline (v2). HW-measured facts from the fp32/fp32r baseline (363us):
PE matmul busy was 270-355us of the span: O matmuls fp32 (N=65, LOW_HIGH
2-pass) 137us, transposes fp32 (2cyc/row) 75us, S junk-pair fp32r 58us.
All three shrink with bf16 operands (1 cyc/row streaming + FWL weight
loads), pushing PE under the ~200us DMA floor (64 MiB/core @ ~330GB/s).

Per-core design:
  * Q/K/V loaded via SWDGE (gpsimd) DMA with inline fp32->bf16 cast
    (HWDGE cannot cast); natural ball layout (seq position on partitions).
    Output store on the SP HWDGE ring.
  * Q^T/K^T via packed 2-ball PE transposes of bf16 inputs (1 cyc/row):
    input [128 seq, 2ball x 64d] -> bf16 PSUM [2ball x 64d, 128 seq].
    PSUM->SBUF copies: DVE takes Q^T, ACT takes K^T (bf16 in/out).
  * S^T = K Q^T per ball (no junk pairs): bf16 matmul N=128, contraction
    64 rows at base partition 64*(ball parity) -> consecutive matmuls use
    disjoint row halves and overlap in the PE array; PSUM bank alternates
    with ball parity.
  * E = exp(S^T/8) on ACT, one op per 4-ball group, bf16 out; the
    ones-column appended to V provides softmax denominators through the
    O matmul; normalize on DVE via per-partition reciprocal broadcast.
  * O_unnorm = E^T [V|1] as bf16 matmuls (N=65, 1 cyc/row).
"""

import os
import sys

for _p in ("/opt/trn_rl_repo",):
    if _p not in sys.path and os.path.isdir(_p):
        sys.path.insert(0, _p)

from contextlib import ExitStack

import numpy as np

import concourse.bass as bass
import concourse.mybir as mybir
import concourse.tile as tile
from concourse import bacc
from concourse._compat import with_exitstack
from concourse.masks import make_identity

B, H, N, DH = 4, 16, 8192, 64
BS = 128                 # ball size == SBUF partition count
NCORES = 8
HEADS = B * H // NCORES  # heads per core (8)
M = N // BS              # balls per head (64)

FP32 = mybir.dt.float32
FP32R = mybir.dt.float32r
BF16 = mybir.dt.bfloat16

GRP = 4
NCHUNK = int(os.environ.get("BALL_NCHUNK", "2"))  # head-load split


@with_exitstack
def ball_attention_kernel(
    ctx: ExitStack,
    tc: tile.TileContext,
    out_ap: bass.AP,
    q_ap: bass.AP,
    k_ap: bass.AP,
    v_ap: bass.AP,
    heads: int = HEADS,
    m: int = M,
):
    nc = tc.nc
    assert m % GRP == 0
    ngrp = m // GRP
    scale = 1.0 / float(np.sqrt(DH))

    const_pool = ctx.enter_context(tc.tile_pool(name="const", bufs=1))
    io_pool = ctx.enter_context(tc.tile_pool(name="io", bufs=2))
    t_sb_pool = ctx.enter_context(tc.tile_pool(name="t_sb", bufs=3))
    e_pool = ctx.enter_context(tc.tile_pool(name="e", bufs=2))
    r_pool = ctx.enter_context(tc.tile_pool(name="r", bufs=2))
    t_ps_pool = ctx.enter_context(tc.tile_pool(name="t_ps", bufs=2, space="PSUM"))
    s_ps_pool = ctx.enter_context(tc.tile_pool(name="s_ps", bufs=2, space="PSUM"))
    o_ps_pool = ctx.enter_context(tc.tile_pool(name="o_ps", bufs=2, space="PSUM"))

    ident = const_pool.tile([BS, BS], BF16)
    make_identity(nc, ident)

    mc = m // NCHUNK
    for h in range(heads):
        # ---- loads: SWDGE casts fp32 -> bf16 in the DMA datapath ---------
        q_sb = io_pool.tile([BS, m, DH], BF16, tag="q")
        k_sb = io_pool.tile([BS, m, DH], BF16, tag="k")
        vt = io_pool.tile([BS, m, DH + 1], BF16, tag="vt")
        qv_ = q_ap[h].rearrange("(mm p) d -> p mm d", p=BS)
        kv_ = k_ap[h].rearrange("(mm p) d -> p mm d", p=BS)
        vv_ = v_ap[h].rearrange("(mm p) d -> p mm d", p=BS)
        for c in range(NCHUNK):
            cs = slice(c * mc, (c + 1) * mc)
            nc.gpsimd.dma_start(q_sb[:, cs, :], qv_[:, cs, :])
            nc.gpsimd.dma_start(k_sb[:, cs, :], kv_[:, cs, :])
            nc.gpsimd.dma_start(vt[:, cs, 0:DH], vv_[:, cs, :])
        nc.vector.memset(vt[:, :, DH], 1.0)
        ob = io_pool.tile([BS, m, DH], FP32, tag="ob")

        for g in range(ngrp):
            m0 = g * GRP
            # ---- transposes: packed 2-ball, bf16 in -> bf16 PSUM ---------
            t_ps = t_ps_pool.tile([BS, 4, BS], BF16, tag="t")
            nc.tensor.transpose(t_ps[:, 0, :], q_sb[:, m0 : m0 + 2, :], ident)
            nc.tensor.transpose(t_ps[:, 1, :], k_sb[:, m0 : m0 + 2, :], ident)
            nc.tensor.transpose(t_ps[:, 2, :], q_sb[:, m0 + 2 : m0 + 4, :], ident)
            nc.tensor.transpose(t_ps[:, 3, :], k_sb[:, m0 + 2 : m0 + 4, :], ident)
            qt = t_sb_pool.tile([BS, 2, BS], BF16, tag="qt")  # [pair, seq]
            kt = t_sb_pool.tile([BS, 2, BS], BF16, tag="kt")
            nc.vector.tensor_copy(qt, t_ps[:, 0:4:2, :])
            nc.scalar.copy(kt, t_ps[:, 1:4:2, :])

            # ---- S^T matmuls: per-ball N=128 bf16 ------------------------
            # ball m0+j: pair a2 = j>>1, parity b = j&1; operands live on
            # partitions [64b, 64b+64) -> consecutive matmuls hit different
            # row halves (overlap in PE) and different PSUM banks: dim 1 of
            # s_ps strides a full 2 KiB bank (4*BS fp32), so bank = j%2.
            s_ps = s_ps_pool.tile([BS, 2, 2, 2, BS], FP32, tag="s")
            for j in range(GRP):
                a2, b = j >> 1, j & 1
                lo = 64 * b
                nc.tensor.matmul(
                    s_ps[:, j % 2, j // 2, 0, :],
                    kt[lo : lo + 64, a2, :],
                    qt[lo : lo + 64, a2, :],
                    start=True,
                    stop=True,
                )
            # ---- E = exp(S^T/8): bf16 out; ball m0+j sits at
            # s_ps[:, j%2, j//2, 0, :], so pair a2 spans slots a2*2..a2*2+1.
            e_sb = e_pool.tile([BS, GRP, BS], BF16, tag="e")
            for a2 in range(2):
                nc.scalar.activation(
                    e_sb[:, a2 * 2 : a2 * 2 + 2, :],
                    s_ps[:, :, a2, 0, :],
                    mybir.ActivationFunctionType.Exp,
                    scale=scale,
                )

            # ---- O_unnorm = E^T @ [V | 1] --------------------------------
            o_ps = o_ps_pool.tile([BS, GRP, DH + 1], FP32, tag="o")
            for j in range(GRP):
                nc.tensor.matmul(
                    o_ps[:, j, :],
                    e_sb[:, j, :],
                    vt[:, m0 + j, :],
                    start=True,
                    stop=True,
                )
            # ---- normalize by the ones-column sums -----------------------
            r_sb = r_pool.tile([BS, GRP], FP32, tag="r")
            nc.vector.reciprocal(r_sb, o_ps[:, :, DH])
            nc.vector.tensor_mul(
                ob[:, m0 : m0 + GRP, :],
                o_ps[:, :, 0:DH],
                r_sb.unsqueeze(2).broadcast_to([BS, GRP, DH]),
            )

        # ---- store (SP HWDGE ring; Q7 is busy generating load descs) -----
        nc.sync.dma_start(out_ap[h].rearrange("(mm p) d -> p mm d", p=BS), ob)


def build_nc(heads: int = HEADS, m: int = M):
    nc = bacc.Bacc("TRN2", target_bir_lowering=False, debug=False, num_devices=NCORES)
    q = nc.dram_tensor("q", [heads, m * BS, DH], FP32, kind="ExternalInput").ap()
    k = nc.dram_tensor("k", [heads, m * BS, DH], FP32, kind="ExternalInput").ap()
    v = nc.dram_tensor("v", [heads, m * BS, DH], FP32, kind="ExternalInput").ap()
    o = nc.dram_tensor("out", [heads, m * BS, DH], FP32, kind="ExternalOutput").ap()
    with tile.TileContext(nc) as tc:
        ball_attention_kernel(tc, o, q, k, v, heads=heads, m=m)
    nc.compile()
    return nc


_NC_CACHE = {}


def kernel(q: np.ndarray, k: np.ndarray, v: np.ndarray) -> np.ndarray:
    from concourse.bass_utils import run_bass_kernel_spmd

    assert q.shape == (B, H, N, DH)
    if "nc" not in _NC_CACHE:
        _NC_CACHE["nc"] = build_nc()
    nc = _NC_CACHE["nc"]

    hpc = HEADS
    qf = np.ascontiguousarray(np.asarray(q, dtype=np.float32).reshape(B * H, N, DH))
    kf = np.ascontiguousarray(np.asarray(k, dtype=np.float32).reshape(B * H, N, DH))
    vf = np.ascontiguousarray(np.asarray(v, dtype=np.float32).reshape(B * H, N, DH))
    in_maps = [
        {
            "q": np.ascontiguousarray(qf[c * hpc : (c + 1) * hpc]),
            "k": np.ascontiguousarray(kf[c * hpc : (c + 1) * hpc]),
            "v": np.ascontiguousarray(vf[c * hpc : (c + 1) * hpc]),
        }
        for c in range(NCORES)
    ]
    res = run_bass_kernel_spmd(nc, in_maps, core_ids=list(range(NCORES)))
    out = np.concatenate([res.results[c]["out"] for c in range(NCORES)], axis=0)
    return out.reshape(B, H, N, DH)


# revision 5
# speedup vs baseline: 2.0819x; 1.4234x over previous
"""Ball attention (block-local attention, ball size 128) on 8 Trainium2 cores.

Reference computation (per (b,h) head, per ball of 128 consecutive tokens):
    S = Q K^T / sqrt(64);  P = softmax(S, axis=-1);  O = P V

Sharding: the 64 (b,h) heads are split 8-per-core (pure data parallel).
The host shard step stages each core's inputs in a DMA-friendly tiling
[head, token-in-ball, ball, d] (a pure byte reorder of the same fp32
values; the gather step applies the inverse to the output). V is staged
with a 65th ones column so softmax denominators fall out of the O matmul.

Why: with the natural [head, seq, d] layout, every (partition, ball) pair
is a separate 256-byte DRAM run, so the 64 MiB/core of HBM traffic costs
~262k DMA descriptors at ~18 ns each — measured 84-91% SDMA busy and the
dominant cost. The ball-major tiling makes per-partition runs 16 KiB
(~4k descriptors total) so DMA runs at payload rate.

Per-core compute (HW-measured on the fp32/fp32r baseline and bf16 v2):
  * Loads via SWDGE (gpsimd) DMA with inline fp32->bf16 cast.
  * Q^T/K^T per ball pair as plain matmuls (stationary = 2-ball packed
    [128 seq, 2x64d] bf16 slab, moving = bf16 identity): out fp32 PSUM
    [2x64d, 128 seq]. Plain MM streams at 1 cyc/col vs transpose-mode's
    1.2 GHz path (measured 107ns -> ~55ns/op). PSUM->SBUF copies round
    to bf16: DVE takes Q^T, ACT takes K^T.
  * S^T = K Q^T per ball: bf16 matmul N=128, contraction 64 rows at base
    partition 64*(ball parity): consecutive matmuls hit disjoint row
    halves and overlap in the PE array (measured ~4ns second-of-pair);
    PSUM bank alternates with parity (concurrent same-bank writes fault).
  * E = exp(S^T/8): one ACT op per 4-ball group, bf16 out, slot (b2,a2).
  * O_unnorm = E^T [V|1]: bf16 matmuls N=65 (measured ~54ns/op).
  * Normalize on DVE via per-partition reciprocal broadcast; store fp32
    in the ball-major tiling on the SP HWDGE ring.
"""

import os
import sys

for _p in ("/opt/trn_rl_repo",):
    if _p not in sys.path and os.path.isdir(_p):
        sys.path.insert(0, _p)

from contextlib import ExitStack

import numpy as np

import concourse.bass as bass
import concourse.mybir as mybir
import concourse.tile as tile
from concourse import bacc
from concourse._compat import with_exitstack
from concourse.masks import make_identity

B, H, N, DH = 4, 16, 8192, 64
BS = 128                 # ball size == SBUF partition count
NCORES = 8
HEADS = B * H // NCORES  # heads per core (8)
M = N // BS              # balls per head (64)

FP32 = mybir.dt.float32
BF16 = mybir.dt.bfloat16

GRP = 4
NCHUNK = int(os.environ.get("BALL_NCHUNK", "2"))  # head-load split


@with_exitstack
def ball_attention_kernel(
    ctx: ExitStack,
    tc: tile.TileContext,
    out_ap: bass.AP,
    q_ap: bass.AP,
    k_ap: bass.AP,
    v_ap: bass.AP,
    heads: int = HEADS,
    m: int = M,
):
    nc = tc.nc
    assert m % GRP == 0
    ngrp = m // GRP
    scale = 1.0 / float(np.sqrt(DH))

    const_pool = ctx.enter_context(tc.tile_pool(name="const", bufs=1))
    io_pool = ctx.enter_context(tc.tile_pool(name="io", bufs=2))
    t_sb_pool = ctx.enter_context(tc.tile_pool(name="t_sb", bufs=3))
    e_pool = ctx.enter_context(tc.tile_pool(name="e", bufs=2))
    r_pool = ctx.enter_context(tc.tile_pool(name="r", bufs=2))
    t_ps_pool = ctx.enter_context(tc.tile_pool(name="t_ps", bufs=2, space="PSUM"))
    s_ps_pool = ctx.enter_context(tc.tile_pool(name="s_ps", bufs=2, space="PSUM"))
    o_ps_pool = ctx.enter_context(tc.tile_pool(name="o_ps", bufs=2, space="PSUM"))

    ident = const_pool.tile([BS, BS], BF16)
    make_identity(nc, ident)

    mc = m // NCHUNK
    for h in range(heads):
        # ---- loads: SWDGE casts fp32 -> bf16; ball-major staging means the
        # per-partition DRAM run is a whole [ball, d] row (16 KiB).
        q_sb = io_pool.tile([BS, m, DH], BF16, tag="q")
        k_sb = io_pool.tile([BS, m, DH], BF16, tag="k")
        vt = io_pool.tile([BS, m, DH + 1], BF16, tag="vt")
        qv_ = q_ap[h].rearrange("(p mm) d -> p mm d", p=BS)
        kv_ = k_ap[h].rearrange("(p mm) d -> p mm d", p=BS)
        vv_ = v_ap[h]  # already [BS, m, DH+1] with the host-staged ones col
        for c in range(NCHUNK):
            cs = slice(c * mc, (c + 1) * mc)
            nc.gpsimd.dma_start(q_sb[:, cs, :], qv_[:, cs, :])
            nc.gpsimd.dma_start(k_sb[:, cs, :], kv_[:, cs, :])
            nc.gpsimd.dma_start(vt[:, cs, :], vv_[:, cs, :])
        ob = io_pool.tile([BS, m, DH], FP32, tag="ob")

        for g in range(ngrp):
            m0 = g * GRP
            # ---- transposes: packed 2-ball plain matmuls, fp32 PSUM out --
            t_ps = t_ps_pool.tile([BS, 4, BS], FP32, tag="t")
            nc.tensor.matmul(t_ps[:, 0, :], q_sb[:, m0 : m0 + 2, :], ident, start=True, stop=True)
            nc.tensor.matmul(t_ps[:, 1, :], k_sb[:, m0 : m0 + 2, :], ident, start=True, stop=True)
            nc.tensor.matmul(t_ps[:, 2, :], q_sb[:, m0 + 2 : m0 + 4, :], ident, start=True, stop=True)
            nc.tensor.matmul(t_ps[:, 3, :], k_sb[:, m0 + 2 : m0 + 4, :], ident, start=True, stop=True)
            qt = t_sb_pool.tile([BS, 2, BS], BF16, tag="qt")  # [pair, seq]
            kt = t_sb_pool.tile([BS, 2, BS], BF16, tag="kt")
            nc.vector.tensor_copy(qt, t_ps[:, 0:4:2, :])
            nc.scalar.copy(kt, t_ps[:, 1:4:2, :])

            # ---- S^T matmuls: per-ball N=128 bf16 ------------------------
            # ball m0+j: pair a2 = j>>1, parity b = j&1; operands live on
            # partitions [64b, 64b+64) -> consecutive matmuls hit different
            # row halves and run concurrently, so they must also hit
            # different PSUM banks: dim 1 of s_ps strides a full 2 KiB bank.
            s_ps = s_ps_pool.tile([BS, 2, 2, 2, BS], FP32, tag="s")
            for j in range(GRP):
                a2, b = j >> 1, j & 1
                lo = 64 * b
                nc.tensor.matmul(
                    s_ps[:, b, a2, 0, :],
                    kt[lo : lo + 64, a2, :],
                    qt[lo : lo + 64, a2, :],
                    start=True,
                    stop=True,
                )
            # ---- E = exp(S^T/8): one ACT op per group, bf16, slot (b,a2) -
            e_sb = e_pool.tile([BS, 2, 2, BS], BF16, tag="e")
            nc.scalar.activation(
                e_sb,
                s_ps[:, :, :, 0, :],
                mybir.ActivationFunctionType.Exp,
                scale=scale,
            )

            # ---- O_unnorm = E^T @ [V | 1] --------------------------------
            o_ps = o_ps_pool.tile([BS, GRP, DH + 1], FP32, tag="o")
            for j in range(GRP):
                a2, b = j >> 1, j & 1
                nc.tensor.matmul(
                    o_ps[:, j, :],
                    e_sb[:, b, a2, :],
                    vt[:, m0 + j, :],
                    start=True,
                    stop=True,
                )
            # ---- normalize by the ones-column sums -----------------------
            r_sb = r_pool.tile([BS, GRP], FP32, tag="r")
            nc.vector.reciprocal(r_sb, o_ps[:, :, DH])
            nc.vector.tensor_mul(
                ob[:, m0 : m0 + GRP, :],
                o_ps[:, :, 0:DH],
                r_sb.unsqueeze(2).broadcast_to([BS, GRP, DH]),
            )

        # ---- store: ball-major tiling is contiguous per partition --------
        nc.sync.dma_start(out_ap[h].rearrange("(p mm) d -> p mm d", p=BS), ob)


def build_nc(heads: int = HEADS, m: int = M):
    nc = bacc.Bacc("TRN2", target_bir_lowering=False, debug=False, num_devices=NCORES)
    q = nc.dram_tensor("q", [heads, m * BS, DH], FP32, kind="ExternalInput").ap()
    k = nc.dram_tensor("k", [heads, m * BS, DH], FP32, kind="ExternalInput").ap()
    v = nc.dram_tensor("v", [heads, BS, M, DH + 1], FP32, kind="ExternalInput").ap()
    o = nc.dram_tensor("out", [heads, m * BS, DH], FP32, kind="ExternalOutput").ap()
    with tile.TileContext(nc) as tc:
        ball_attention_kernel(tc, o, q, k, v, heads=heads, m=m)
    nc.compile()
    return nc


_NC_CACHE = {}


def _stage_qk(x: np.ndarray) -> np.ndarray:
    """[heads, N, DH] fp32 -> ball-major [heads, BS*M, DH] (token, ball, d)."""
    hp = x.shape[0]
    return np.ascontiguousarray(
        x.reshape(hp, M, BS, DH).transpose(0, 2, 1, 3).reshape(hp, N, DH)
    )


def _stage_v(x: np.ndarray) -> np.ndarray:
    """[heads, N, DH] fp32 -> ball-major [heads, BS, M, DH+1] with ones col."""
    hp = x.shape[0]
    out = np.empty((hp, BS, M, DH + 1), dtype=np.float32)
    out[..., :DH] = x.reshape(hp, M, BS, DH).transpose(0, 2, 1, 3)
    out[..., DH] = 1.0
    return out


def kernel(q: np.ndarray, k: np.ndarray, v: np.ndarray) -> np.ndarray:
    from concourse.bass_utils import run_bass_kernel_spmd

    assert q.shape == (B, H, N, DH)
    if "nc" not in _NC_CACHE:
        _NC_CACHE["nc"] = build_nc()
    nc = _NC_CACHE["nc"]

    hpc = HEADS
    qf = np.asarray(q, dtype=np.float32).reshape(B * H, N, DH)
    kf = np.asarray(k, dtype=np.float32).reshape(B * H, N, DH)
    vf = np.asarray(v, dtype=np.float32).reshape(B * H, N, DH)
    in_maps = [
        {
            "q": _stage_qk(qf[c * hpc : (c + 1) * hpc]),
            "k": _stage_qk(kf[c * hpc : (c + 1) * hpc]),
            "v": _stage_v(vf[c * hpc : (c + 1) * hpc]),
        }
        for c in range(NCORES)
    ]
    res = run_bass_kernel_spmd(nc, in_maps, core_ids=list(range(NCORES)))
    out = np.concatenate([res.results[c]["out"] for c in range(NCORES)], axis=0)
    # un-permute: device wrote [head, token-in-ball, ball, d]
    out = out.reshape(B * H, BS, M, DH).transpose(0, 2, 1, 3)
    return np.ascontiguousarray(out).reshape(B, H, N, DH)
